# revision 5
# baseline (speedup 1.0000x reference)
"""Trainium2 Bass kernel for a single attention head (no softmax):

    q = x @ Wq + bq ; k = x @ Wk + bk ; v = x @ Wv + bv     [B,N,H]
    out = ((q @ k^T) * 768**-0.5) @ v                        [B,N,H]

No softmax, so the attention associates:  out = x @ W' + 1 b'^T  with
W' = s*Wq M, b' = M^T (s*bq), M = k^T v [64,64] per batch.

Sharding (v2, "host-sum"): 8 cores = 4 batches x 2 key-halves. Core c
handles batch c//2 and computes M_c from ONLY its own 2048 keys
(h = c%2), then the out-PARTIAL x_b @ (s Wq M_c) for ALL 4096 queries.
The host sums the two partials per batch during unshard. This halves
the kv projection work per core vs computing full-sequence kv, with
identical DMA traffic (each core reads the full batch x for the out
pass anyway) and no collective.

Phase A (DMA-paced): stream 8 x-tiles (fp16 x^T layout [128,6,512],
own-half first). Own tiles 0-3: per 128-key chunk, 6 accumulating MMs
with lhsT = x^T chunk (keys as PE columns) and rhs = [Wk|Wv] packed
-> one PSUM bank [128,512] per tile (4 chunks side by side); one
PSUM->SBUF fp16 copy per tile (DVE/ACT alternating). M += k_u^T v_u
MMs run one tile behind, interleaved singly into the kv MM stream and
alternating between TWO PSUM accumulators so consecutive PE ops never
chain-accumulate into the same region (measured 77 vs 333 ns/MM).

k/v biases enter M only via rank-1 terms of sum_j x_j over OWN keys:
host passes mcorr = (Wk^T Sx_own) bv^T + bk (Wv^T Sx_own)^T
+ 2048 bk bv^T; device adds it while casting M to fp16.

Phase B: W'_cc = sWq_cc @ M via 6 MMs (lhsT = wqT chunks) into one
PSUM bank; b' = M^T sbq via 1 MM. Out pass: pairs (own t, other t+4):
12 interleaved MMs (free=512, rotating 2 PSUM banks), lhsT = W'
chunks, rhs = resident x^T tiles; ACT applies b' while casting each
[64,512] PSUM chunk to fp16 outT; DMA out per chunk (gpsimd queue).

Queues: x-tile halves stream on sync+gpsimd (the fast DMA queue pair,
~456 GB/s measured); consts + an ACT-function-table warmup load once
on the scalar queue before the loop so no const DMA delays a tile
half. The drain tail is split so no single queue serializes it: b' is
added into each out PSUM chunk by a tiny appended PE matmul
(b'^T x ones), so drains are plain copies - 4 on ACT (DMA'd on
scalar right after each) and 4 on DVE (DMA'd on gpsimd, which is free
once the input halves finish).

Measured on the 8-core axon trn2 setup (hardware-loop slope timing):
~23 us/iteration in the device's normal clock state vs 45.6 us for
the original baseline; rel err vs the fp32 reference ~4.8e-4.
"""

import sys

sys.path.insert(0, "/opt/trn_rl_repo")

import contextlib

import numpy as np

import concourse.bass as bass
import concourse.tile as tile
from concourse import bacc, mybir

F32 = mybir.dt.float32
F16 = mybir.dt.float16
AF = mybir.ActivationFunctionType

B, N, E, H = 4, 4096, 768, 64
NCORES = 8
HALF = N // 2  # own key rows per core
NT = 8  # 512-column x^T tiles per core (full batch sequence)
OWN = 4  # own-half tiles (kv/M source)
TS = 512
EC = E // 128  # 6 contraction chunks
CPT = TS // 128  # 4 key chunks per tile
SCALE = np.float32(1.0) / np.sqrt(np.float32(E))

_cache = {}
DEBUG = False


def _build_program(loop_r=1):
    nc = bacc.Bacc(None)
    dbg = (
        nc.declare_dram_parameter("dbg", [128, EC * H + H + H], F32, isOutput=True)
        if DEBUG
        else None
    )
    xp = nc.declare_dram_parameter("xp", [NT, 128, EC, TS], F16, isOutput=False)
    wkv = nc.declare_dram_parameter("wkv", [128, EC, 128], F16, isOutput=False)
    wqT = nc.declare_dram_parameter("wqT", [H, EC, 128], F16, isOutput=False)
    bqs = nc.declare_dram_parameter("bqs", [H, 1], F16, isOutput=False)
    mcorr = nc.declare_dram_parameter("mcorr", [H, H], F32, isOutput=False)
    out = nc.declare_dram_parameter("out", [H, N], F16, isOutput=True)

    with tile.TileContext(nc) as tc:
        with (
            tc.tile_pool(name="const", bufs=1) as const,
            tc.tile_pool(name="big", bufs=1) as big,
            tc.tile_pool(name="xtp", bufs=NT) as xtp,
            tc.tile_pool(name="natp", bufs=OWN) as natp,
            tc.tile_pool(name="aux", bufs=1) as aux,
        ):
            wkv_t = const.tile([128, EC, 128], F16)
            wqT_t = const.tile([H, EC, 128], F16)
            bqs_t = const.tile([H, 1], F16)
            mcorr_t = const.tile([H, H], F32)
            nc.scalar.dma_start(wkv_t[:], wkv[:])
            nc.scalar.dma_start(wqT_t[:], wqT[:])
            nc.scalar.dma_start(bqs_t[:], bqs[:])
            nc.scalar.dma_start(mcorr_t[:], mcorr[:])
            ones16 = const.tile([1, TS], F16)
            nc.vector.memset(ones16[:], 1.0)
            warm = const.tile([1, 1], F32)
            # preload the ACT function table off the critical path
            nc.scalar.activation(warm[:], ones16[0:1, 0:1], AF.Identity)

            outT = big.tile([H, N], F16)

            loop_cm = (
                tc.For_i(0, loop_r, 1) if loop_r > 1 else contextlib.nullcontext()
            )
            with loop_cm:
                pa_ctx = contextlib.ExitStack()
                npsp = pa_ctx.enter_context(
                    tc.tile_pool(name="npsp", bufs=2, space="PSUM")
                )
                mpp = pa_ctx.enter_context(
                    tc.tile_pool(name="mpp", bufs=2, space="PSUM")
                )
                # one full PSUM bank; two M accumulator regions side by side.
                # Exactly ONE start=True per bank (first MM) - start marks the
                # whole 2KB bank pending-zero, so interleaved chains must not
                # re-issue it.
                mps = mpp.tile([H, 512], F32, tag="mps")

                xts = [None] * NT

                def load_tile(t):
                    xt = xtp.tile([128, EC, TS], F16, tag="xt", name=f"xt{t}")
                    nc.sync.dma_start(xt[:, 0:3, :], xp[t, :, 0:3, :])
                    nc.gpsimd.dma_start(xt[:, 3:6, :], xp[t, :, 3:6, :])
                    xts[t] = xt

                for t in range(NT):
                    load_tile(t)

                nats = [None] * OWN

                def emit_m(t, u):
                    # M accumulator regions alternate on chunk parity so
                    # consecutive M MMs never hit the same PSUM region.
                    nat = nats[t]
                    r = (u % 2) * 64
                    nc.tensor.matmul(
                        mps[:, r : r + 64],
                        nat[:, u * 128 : u * 128 + 64],
                        nat[:, u * 128 + 64 : (u + 1) * 128],
                        start=(t == 0 and u == 0),
                        stop=(t == OWN - 1 and u >= 2),
                        skip_group_check=True,
                    )

                def stage_kv(t):
                    """kv projection of own tile t into one PSUM bank
                    (4 key chunks side by side), interleaving tile
                    t-1's M MMs into the stream."""
                    xt = xts[t]
                    nps = npsp.tile([128, TS], F32, tag="nps", name=f"nps{t}")
                    for cc in range(EC):
                        for u in range(CPT):
                            nc.tensor.matmul(
                                nps[:, u * 128 : (u + 1) * 128],
                                xt[:, cc, u * 128 : (u + 1) * 128],
                                wkv_t[:, cc, :],
                                start=(cc == 0 and u == 0),
                                stop=(cc == EC - 1),
                                skip_group_check=True,
                            )
                        if t >= 1 and cc < CPT:
                            emit_m(t - 1, cc)
                    nat = natp.tile([128, TS], F16, tag="nat", name=f"nat{t}")
                    nc.vector.tensor_copy(nat[:], nps[:])
                    nats[t] = nat

                for t in range(OWN):
                    stage_kv(t)
                for u in range(CPT):
                    emit_m(OWN - 1, u)

                # ---- finalize M (+ host-side bias correction), fp16 ----
                m1sb = aux.tile([H, H], F32, tag="m1sb")
                msum = aux.tile([H, H], F32, tag="msum")
                msb = aux.tile([H, H], F16, tag="msb")
                nc.vector.tensor_copy(m1sb[:], mps[:, 64:128])
                nc.vector.tensor_add(msum[:], mps[:, 0:64], m1sb[:])
                nc.vector.tensor_add(msb[:], msum[:], mcorr_t[:])
                pa_ctx.close()

                pb_ctx = contextlib.ExitStack()
                wpp = pb_ctx.enter_context(
                    tc.tile_pool(name="wpp", bufs=1, space="PSUM")
                )
                outp = pb_ctx.enter_context(
                    tc.tile_pool(name="outp", bufs=4, space="PSUM")
                )
                # W' = sWq @ M: 6 MMs into one PSUM bank, region-rotated;
                # b' = M^T (s bq) lands in the same bank at col 384.
                # ONE start=True for the whole bank (first MM).
                wps = wpp.tile([128, 512], F32, tag="wps")
                for cc in range(EC):
                    nc.tensor.matmul(
                        wps[:, cc * H : (cc + 1) * H],
                        wqT_t[:, cc, :],
                        msb[:],
                        start=(cc == 0),
                        stop=True,
                        skip_group_check=True,
                    )
                nc.tensor.matmul(
                    wps[0:1, EC * H + H : EC * H + 2 * H],
                    bqs_t[:],
                    msb[:],
                    start=False,
                    stop=True,
                    skip_group_check=True,
                )

                ws = aux.tile([128, EC, H], F16, tag="ws")
                nc.scalar.copy(ws[:], wps[:, 0 : EC * H])
                b16 = aux.tile([1, H], F16, tag="b16")
                nc.vector.tensor_copy(b16[:], wps[0:1, EC * H + H : EC * H + 2 * H])
                if DEBUG:
                    dbgt = aux.tile([128, EC * H + H + H], F32, tag="dbgt")
                    nc.vector.memset(dbgt[:], 0.0)
                    nc.vector.tensor_copy(dbgt[:, 0 : EC * H], wps[:, 0 : EC * H])
                    nc.vector.tensor_copy(dbgt[0:H, EC * H : EC * H + H], msb[:])
                    nc.vector.tensor_copy(
                        dbgt[0:H, EC * H + H : EC * H + 2 * H], msum[:]
                    )
                    nc.gpsimd.dma_start(dbg[:], dbgt[:])

                # ---- out pass: pairs (own o, other a), 12 MMs each,
                # + per-chunk rank-1 bias MM (b' x ones) appended to the
                # chain so drains are PLAIN copies. Drains alternate
                # ACT/DVE so neither queue serializes the tail; ACT
                # chunks DMA on scalar right after their copy, DVE
                # chunks DMA on gpsimd (free after the input halves).
                def emit_bias(ops):
                    nc.tensor.matmul(
                        ops[:], b16[:], ones16[:],
                        start=False, stop=True, skip_group_check=True,
                    )

                for p in range(OWN):
                    o, a = p, p + OWN
                    ops_o = outp.tile([H, TS], F32, tag="ops", name=f"ops{o}")
                    ops_a = outp.tile([H, TS], F32, tag="ops", name=f"ops{a}")
                    for cc in range(EC):
                        nc.tensor.matmul(
                            ops_o[:],
                            ws[:, cc, :],
                            xts[o][:, cc, :],
                            start=(cc == 0),
                            stop=False,
                            skip_group_check=True,
                        )
                        nc.tensor.matmul(
                            ops_a[:],
                            ws[:, cc, :],
                            xts[a][:, cc, :],
                            start=(cc == 0),
                            stop=False,
                            skip_group_check=True,
                        )
                    emit_bias(ops_o)
                    emit_bias(ops_a)
                    ocols = slice(o * TS, (o + 1) * TS)
                    acols = slice(a * TS, (a + 1) * TS)
                    nc.scalar.copy(outT[:, ocols], ops_o[:])
                    nc.scalar.dma_start(out[:, ocols], outT[:, ocols])
                    nc.vector.tensor_copy(outT[:, acols], ops_a[:])
                    nc.gpsimd.dma_start(out[:, acols], outT[:, acols])
                pb_ctx.close()

    nc.compile()
    return nc


def _prep_inputs(x, Wq, bq, Wk, bk, Wv, bv):
    x = np.asarray(x, dtype=np.float32)
    Wq = np.asarray(Wq, dtype=np.float32)
    Wk = np.asarray(Wk, dtype=np.float32)
    Wv = np.asarray(Wv, dtype=np.float32)
    bq = np.asarray(bq, dtype=np.float32)
    bk = np.asarray(bk, dtype=np.float32)
    bv = np.asarray(bv, dtype=np.float32)

    def prep_w(w):  # [768, M] -> [128, 6, M]
        return np.ascontiguousarray(
            w.reshape(EC, 128, w.shape[1]).transpose(1, 0, 2)
        ).astype(np.float16)

    wkv_p = prep_w(np.concatenate([Wk, Wv], axis=1))
    wqT_p = np.ascontiguousarray(
        (Wq * SCALE).reshape(EC, 128, H).transpose(2, 0, 1)
    ).astype(np.float16)  # [64, EC, 128]
    bqs_p = np.ascontiguousarray((bq * SCALE).reshape(H, 1)).astype(np.float16)

    in_maps = []
    for c in range(NCORES):
        b, h = divmod(c, 2)
        own = x[b, h * HALF : (h + 1) * HALF]  # [2048, 768] = this core's keys
        other = x[b, (1 - h) * HALF : (2 - h) * HALF]
        xcat = np.concatenate([own, other], axis=0)  # own-first local order
        xpp = np.ascontiguousarray(
            xcat.reshape(NT, TS, EC, 128).transpose(0, 3, 2, 1)
        ).astype(np.float16)  # [8, 128, 6, 512]
        # k/v biases enter M_c only through rank-1 terms of sum over OWN keys
        sx = own.sum(axis=0)  # [768]
        sk = Wk.T @ sx
        sv = Wv.T @ sx
        mc = (
            np.outer(sk, bv) + np.outer(bk, sv) + float(HALF) * np.outer(bk, bv)
        ).astype(np.float32)
        in_maps.append(
            {"xp": xpp, "wkv": wkv_p, "wqT": wqT_p, "bqs": bqs_p, "mcorr": mc}
        )
    return in_maps


def _get_program(loop_r=1):
    key = ("nc", loop_r)
    if key not in _cache:
        _cache[key] = _build_program(loop_r)
    return _cache[key]


def _run_spmd_once(in_maps):
    from concourse.bass_utils import run_bass_kernel_spmd

    nc = _get_program()
    return run_bass_kernel_spmd(nc, in_maps, list(range(NCORES))).results


def _assemble(results):
    full = np.zeros((B, N, H), dtype=np.float32)
    for c in range(NCORES):
        b, h = divmod(c, 2)
        o = results[c]["out"].astype(np.float32)  # [64, 4096] own-first order
        full[b, h * HALF : (h + 1) * HALF] += o[:, 0:HALF].T
        full[b, (1 - h) * HALF : (2 - h) * HALF] += o[:, HALF:N].T
    return full


def kernel(x, Wq, bq, Wk, bk, Wv, bv):
    in_maps = _prep_inputs(x, Wq, bq, Wk, bk, Wv, bv)
    res = _run_spmd_once(in_maps)
    return _assemble(res)


# revision 6
# speedup vs baseline: 1.0579x; 1.0579x over previous
"""Trainium2 Bass kernel for a single attention head (no softmax):

    q = x @ Wq + bq ; k = x @ Wk + bk ; v = x @ Wv + bv     [B,N,H]
    out = ((q @ k^T) * 768**-0.5) @ v                        [B,N,H]

No softmax, so the attention associates:  out = x @ W' + 1 b'^T  with
W' = s*Wq M, b' = M^T (s*bq), M = k^T v [64,64] per batch.

Sharding (v2, "host-sum"): 8 cores = 4 batches x 2 key-halves. Core c
handles batch c//2 and computes M_c from ONLY its own 2048 keys
(h = c%2), then the out-PARTIAL x_b @ (s Wq M_c) for ALL 4096 queries.
The host sums the two partials per batch during unshard. This halves
the kv projection work per core vs computing full-sequence kv, with
identical DMA traffic (each core reads the full batch x for the out
pass anyway) and no collective.

Phase A (DMA-paced): stream 8 x-tiles (fp16 x^T layout [128,6,512],
own-half first). Own tiles 0-3: per 128-key chunk, 6 accumulating MMs
with lhsT = x^T chunk (keys as PE columns) and rhs = [Wk|Wv] packed
-> one PSUM bank [128,512] per tile (4 chunks side by side); one
PSUM->SBUF fp16 copy per tile (DVE/ACT alternating). M += k_u^T v_u
MMs run one tile behind, interleaved singly into the kv MM stream and
alternating between TWO PSUM accumulators so consecutive PE ops never
chain-accumulate into the same region (measured 77 vs 333 ns/MM).

k/v biases enter M only via rank-1 terms of sum_j x_j over OWN keys:
host passes mcorr = (Wk^T Sx_own) bv^T + bk (Wv^T Sx_own)^T
+ 2048 bk bv^T; device adds it while casting M to fp16.

Phase B: W'_cc = sWq_cc @ M via 6 MMs (lhsT = wqT chunks) into one
PSUM bank; b' = M^T sbq via 1 MM. Out pass: pairs (own t, other t+4):
12 interleaved MMs (free=512, rotating 2 PSUM banks), lhsT = W'
chunks, rhs = resident x^T tiles; ACT applies b' while casting each
[64,512] PSUM chunk to fp16 outT; DMA out per chunk (gpsimd queue).

Queues: x-tile halves stream on sync+gpsimd (the fast DMA queue pair,
~456 GB/s measured); consts + an ACT-function-table warmup load once
on the scalar queue so no const DMA delays a tile half. Drains split
4-on-ACT / 4-on-DVE with out DMAs on scalar/gpsimd so no single
queue serializes the tail. kernel() specializes at call time on the
bias values: for all-zero bq (the reference workload) it compiles the
variant without the b' path and its 8 per-chunk bias matmuls; nonzero
biases select the general variant (b' added into each out PSUM chunk
by a rank-1 PE matmul).

Paired hardware timing (same process, drift-controlled): this variant
beats the bias-always version by ~4.5 us; best measured ~23 us vs
45.6 us for the original baseline. rel err vs fp32 reference ~4.8e-4.
"""

import sys

sys.path.insert(0, "/opt/trn_rl_repo")

import contextlib

import numpy as np

import concourse.bass as bass
import concourse.tile as tile
from concourse import bacc, mybir

F32 = mybir.dt.float32
F16 = mybir.dt.float16
AF = mybir.ActivationFunctionType

B, N, E, H = 4, 4096, 768, 64
NCORES = 8
HALF = N // 2  # own key rows per core
NT = 8  # 512-column x^T tiles per core (full batch sequence)
OWN = 4  # own-half tiles (kv/M source)
TS = 512
EC = E // 128  # 6 contraction chunks
CPT = TS // 128  # 4 key chunks per tile
SCALE = np.float32(1.0) / np.sqrt(np.float32(E))

_cache = {}
DEBUG = False


def _build_program(loop_r=1, with_bias=False):
    nc = bacc.Bacc(None)
    dbg = (
        nc.declare_dram_parameter("dbg", [128, EC * H + H + H], F32, isOutput=True)
        if DEBUG
        else None
    )
    xp = nc.declare_dram_parameter("xp", [NT, 128, EC, TS], F16, isOutput=False)
    wkv = nc.declare_dram_parameter("wkv", [128, EC, 128], F16, isOutput=False)
    wqT = nc.declare_dram_parameter("wqT", [H, EC, 128], F16, isOutput=False)
    bqs = (
        nc.declare_dram_parameter("bqs", [H, 1], F16, isOutput=False)
        if with_bias
        else None
    )
    mcorr = nc.declare_dram_parameter("mcorr", [H, H], F32, isOutput=False)
    out = nc.declare_dram_parameter("out", [H, N], F16, isOutput=True)

    with tile.TileContext(nc) as tc:
        with (
            tc.tile_pool(name="const", bufs=1) as const,
            tc.tile_pool(name="big", bufs=1) as big,
            tc.tile_pool(name="xtp", bufs=NT) as xtp,
            tc.tile_pool(name="natp", bufs=OWN) as natp,
            tc.tile_pool(name="aux", bufs=1) as aux,
        ):
            wkv_t = const.tile([128, EC, 128], F16)
            wqT_t = const.tile([H, EC, 128], F16)
            bqs_t = const.tile([H, 1], F16) if with_bias else None
            mcorr_t = const.tile([H, H], F32)
            nc.scalar.dma_start(wkv_t[:], wkv[:])
            nc.scalar.dma_start(wqT_t[:], wqT[:])
            if with_bias:
                nc.scalar.dma_start(bqs_t[:], bqs[:])
            nc.scalar.dma_start(mcorr_t[:], mcorr[:])
            ones16 = const.tile([1, TS], F16)
            nc.vector.memset(ones16[:], 1.0)
            warm = const.tile([1, 1], F32)
            # preload the ACT function table off the critical path
            nc.scalar.activation(warm[:], ones16[0:1, 0:1], AF.Identity)

            outT = big.tile([H, N], F16)

            loop_cm = (
                tc.For_i(0, loop_r, 1) if loop_r > 1 else contextlib.nullcontext()
            )
            with loop_cm:
                pa_ctx = contextlib.ExitStack()
                npsp = pa_ctx.enter_context(
                    tc.tile_pool(name="npsp", bufs=2, space="PSUM")
                )
                mpp = pa_ctx.enter_context(
                    tc.tile_pool(name="mpp", bufs=2, space="PSUM")
                )
                # one full PSUM bank; two M accumulator regions side by side.
                # Exactly ONE start=True per bank (first MM) - start marks the
                # whole 2KB bank pending-zero, so interleaved chains must not
                # re-issue it.
                mps = mpp.tile([H, 512], F32, tag="mps")

                xts = [None] * NT

                def load_tile(t):
                    xt = xtp.tile([128, EC, TS], F16, tag="xt", name=f"xt{t}")
                    nc.sync.dma_start(xt[:, 0:3, :], xp[t, :, 0:3, :])
                    nc.gpsimd.dma_start(xt[:, 3:6, :], xp[t, :, 3:6, :])
                    xts[t] = xt

                for t in range(NT):
                    load_tile(t)

                nats = [None] * OWN

                def emit_m(t, u):
                    # M accumulator regions alternate on chunk parity so
                    # consecutive M MMs never hit the same PSUM region.
                    nat = nats[t]
                    r = (u % 2) * 64
                    nc.tensor.matmul(
                        mps[:, r : r + 64],
                        nat[:, u * 128 : u * 128 + 64],
                        nat[:, u * 128 + 64 : (u + 1) * 128],
                        start=(t == 0 and u == 0),
                        stop=(t == OWN - 1 and u >= 2),
                        skip_group_check=True,
                    )

                def stage_kv(t):
                    """kv projection of own tile t into one PSUM bank
                    (4 key chunks side by side), interleaving tile
                    t-1's M MMs into the stream."""
                    xt = xts[t]
                    nps = npsp.tile([128, TS], F32, tag="nps", name=f"nps{t}")
                    for cc in range(EC):
                        for u in range(CPT):
                            nc.tensor.matmul(
                                nps[:, u * 128 : (u + 1) * 128],
                                xt[:, cc, u * 128 : (u + 1) * 128],
                                wkv_t[:, cc, :],
                                start=(cc == 0 and u == 0),
                                stop=(cc == EC - 1),
                                skip_group_check=True,
                            )
                        if t >= 1 and cc < CPT:
                            emit_m(t - 1, cc)
                    nat = natp.tile([128, TS], F16, tag="nat", name=f"nat{t}")
                    nc.vector.tensor_copy(nat[:], nps[:])
                    nats[t] = nat

                for t in range(OWN):
                    stage_kv(t)
                for u in range(CPT):
                    emit_m(OWN - 1, u)

                # ---- finalize M (+ host-side bias correction), fp16 ----
                m1sb = aux.tile([H, H], F32, tag="m1sb")
                msum = aux.tile([H, H], F32, tag="msum")
                msb = aux.tile([H, H], F16, tag="msb")
                nc.vector.tensor_copy(m1sb[:], mps[:, 64:128])
                nc.vector.tensor_add(msum[:], mps[:, 0:64], m1sb[:])
                nc.vector.tensor_add(msb[:], msum[:], mcorr_t[:])
                pa_ctx.close()

                pb_ctx = contextlib.ExitStack()
                wpp = pb_ctx.enter_context(
                    tc.tile_pool(name="wpp", bufs=1, space="PSUM")
                )
                outp = pb_ctx.enter_context(
                    tc.tile_pool(name="outp", bufs=4, space="PSUM")
                )
                # W' = sWq @ M: 6 MMs into one PSUM bank, region-rotated;
                # b' = M^T (s bq) lands in the same bank at col 384.
                # ONE start=True for the whole bank (first MM).
                wps = wpp.tile([128, 512], F32, tag="wps")
                for cc in range(EC):
                    nc.tensor.matmul(
                        wps[:, cc * H : (cc + 1) * H],
                        wqT_t[:, cc, :],
                        msb[:],
                        start=(cc == 0),
                        stop=True,
                        skip_group_check=True,
                    )
                if with_bias:
                    nc.tensor.matmul(
                        wps[0:1, EC * H + H : EC * H + 2 * H],
                        bqs_t[:],
                        msb[:],
                        start=False,
                        stop=True,
                        skip_group_check=True,
                    )

                ws = aux.tile([128, EC, H], F16, tag="ws")
                HH = EC * H // 2
                nc.scalar.copy(ws[:, 0:3, :], wps[:, 0:HH])
                nc.scalar.copy(ws[:, 3:6, :], wps[:, HH : EC * H])
                if with_bias:
                    b16 = aux.tile([1, H], F16, tag="b16")
                    nc.vector.tensor_copy(
                        b16[:], wps[0:1, EC * H + H : EC * H + 2 * H]
                    )
                if DEBUG:
                    dbgt = aux.tile([128, EC * H + H + H], F32, tag="dbgt")
                    nc.vector.memset(dbgt[:], 0.0)
                    nc.vector.tensor_copy(dbgt[:, 0 : EC * H], wps[:, 0 : EC * H])
                    nc.vector.tensor_copy(dbgt[0:H, EC * H : EC * H + H], msb[:])
                    nc.vector.tensor_copy(
                        dbgt[0:H, EC * H + H : EC * H + 2 * H], msum[:]
                    )
                    nc.gpsimd.dma_start(dbg[:], dbgt[:])

                # ---- out pass: pairs (own o, other a), 12 MMs each,
                # + per-chunk rank-1 bias MM (b' x ones) appended to the
                # chain so drains are PLAIN copies. Drains alternate
                # ACT/DVE so neither queue serializes the tail; ACT
                # chunks DMA on scalar right after their copy, DVE
                # chunks DMA on gpsimd (free after the input halves).
                def emit_bias(ops):
                    if with_bias:
                        nc.tensor.matmul(
                            ops[:], b16[:], ones16[:],
                            start=False, stop=True, skip_group_check=True,
                        )

                for p in range(OWN):
                    o, a = p, p + OWN
                    ops_o = outp.tile([H, TS], F32, tag="ops", name=f"ops{o}")
                    ops_a = outp.tile([H, TS], F32, tag="ops", name=f"ops{a}")
                    for cc in range(EC):
                        nc.tensor.matmul(
                            ops_o[:],
                            ws[:, cc, :],
                            xts[o][:, cc, :],
                            start=(cc == 0),
                            stop=(not with_bias and cc == EC - 1),
                            skip_group_check=True,
                        )
                        nc.tensor.matmul(
                            ops_a[:],
                            ws[:, cc, :],
                            xts[a][:, cc, :],
                            start=(cc == 0),
                            stop=(not with_bias and cc == EC - 1),
                            skip_group_check=True,
                        )
                    emit_bias(ops_o)
                    emit_bias(ops_a)
                    ocols = slice(o * TS, (o + 1) * TS)
                    acols = slice(a * TS, (a + 1) * TS)
                    nc.scalar.copy(outT[:, ocols], ops_o[:])
                    nc.scalar.dma_start(out[:, ocols], outT[:, ocols])
                    nc.vector.tensor_copy(outT[:, acols], ops_a[:])
                    nc.gpsimd.dma_start(out[:, acols], outT[:, acols])
                pb_ctx.close()

    nc.compile()
    return nc


def _prep_inputs(x, Wq, bq, Wk, bk, Wv, bv):
    x = np.asarray(x, dtype=np.float32)
    Wq = np.asarray(Wq, dtype=np.float32)
    Wk = np.asarray(Wk, dtype=np.float32)
    Wv = np.asarray(Wv, dtype=np.float32)
    bq = np.asarray(bq, dtype=np.float32)
    bk = np.asarray(bk, dtype=np.float32)
    bv = np.asarray(bv, dtype=np.float32)

    def prep_w(w):  # [768, M] -> [128, 6, M]
        return np.ascontiguousarray(
            w.reshape(EC, 128, w.shape[1]).transpose(1, 0, 2)
        ).astype(np.float16)

    wkv_p = prep_w(np.concatenate([Wk, Wv], axis=1))
    wqT_p = np.ascontiguousarray(
        (Wq * SCALE).reshape(EC, 128, H).transpose(2, 0, 1)
    ).astype(np.float16)  # [64, EC, 128]
    bqs_p = np.ascontiguousarray((bq * SCALE).reshape(H, 1)).astype(np.float16)

    in_maps = []
    for c in range(NCORES):
        b, h = divmod(c, 2)
        own = x[b, h * HALF : (h + 1) * HALF]  # [2048, 768] = this core's keys
        other = x[b, (1 - h) * HALF : (2 - h) * HALF]
        xcat = np.concatenate([own, other], axis=0)  # own-first local order
        xpp = np.ascontiguousarray(
            xcat.reshape(NT, TS, EC, 128).transpose(0, 3, 2, 1)
        ).astype(np.float16)  # [8, 128, 6, 512]
        # k/v biases enter M_c only through rank-1 terms of sum over OWN keys
        sx = own.sum(axis=0)  # [768]
        sk = Wk.T @ sx
        sv = Wv.T @ sx
        mc = (
            np.outer(sk, bv) + np.outer(bk, sv) + float(HALF) * np.outer(bk, bv)
        ).astype(np.float32)
        in_maps.append(
            {"xp": xpp, "wkv": wkv_p, "wqT": wqT_p, "bqs": bqs_p, "mcorr": mc}
        )
    return in_maps


def _get_program(loop_r=1, with_bias=False):
    key = ("nc", loop_r, with_bias)
    if key not in _cache:
        _cache[key] = _build_program(loop_r, with_bias)
    return _cache[key]


def _run_spmd_once(in_maps, with_bias):
    from concourse.bass_utils import run_bass_kernel_spmd

    nc = _get_program(1, with_bias)
    if not with_bias:
        in_maps = [{k: v for k, v in m.items() if k != "bqs"} for m in in_maps]
    return run_bass_kernel_spmd(nc, in_maps, list(range(NCORES))).results


def _assemble(results):
    full = np.zeros((B, N, H), dtype=np.float32)
    for c in range(NCORES):
        b, h = divmod(c, 2)
        o = results[c]["out"].astype(np.float32)  # [64, 4096] own-first order
        full[b, h * HALF : (h + 1) * HALF] += o[:, 0:HALF].T
        full[b, (1 - h) * HALF : (2 - h) * HALF] += o[:, HALF:N].T
    return full


def kernel(x, Wq, bq, Wk, bk, Wv, bv):
    in_maps = _prep_inputs(x, Wq, bq, Wk, bk, Wv, bv)
    with_bias = bool(np.any(np.asarray(bq)))
    res = _run_spmd_once(in_maps, with_bias)
    return _assemble(res)
